# revision 37
# baseline (speedup 1.0000x reference)
"""Causal self-attention (B=2, T=2048, D=1024, H=16, DH=64) on 8 trn2 cores.

Sharding: DP on batch (2) x TP on heads (4 heads/core). Each core computes
qkv for its heads from x[b]^T, RoPE, causal SDPA, and a partial row-parallel
output projection y^T [D, T]. Host sums TP partials, transposes, adds bias.

All matmuls run as float32r (fp32 rounded to 11 mantissa bits, full PE rate).
Everything on-device works in transposed orientation so no device transposes
are needed; the only cross-partition data movement is done on the PE (a
pair-swap permutation matmul for RoPE and a ones-outer-product broadcast for
the softmax normalizer).
"""
import sys

if "/opt/trn_rl_repo" not in sys.path:
    sys.path.insert(0, "/opt/trn_rl_repo")

import numpy as np

B, T, D = 2, 2048, 1024
H, DH = 16, 64
ROPE_BASE = 10000.0
NCORES = 8
TP = 4                # TP group size (cores per batch)
HL = H // TP          # heads per core = 4
CHUNK = 512           # t/q chunk
NCH = T // CHUNK      # 4
KT = 128              # k tile
NKT = T // KT         # 16
DIN = HL * DH         # 256 local head dims
NEG = -1.0e30
SCALE = 1.0 / float(np.sqrt(DH))

_compiled = None
_last_results = None


def _round_fp32r(x: np.ndarray) -> np.ndarray:
    u = np.ascontiguousarray(x, dtype=np.float32).view(np.uint32)
    u = (u + np.uint32(0x7FF) + ((u >> np.uint32(12)) & np.uint32(1))) & np.uint32(0xFFFFF000)
    return u.view(np.float32)


def _build(debug=False, stages=("qkv", "rope", "attn", "proj")):
    import concourse.bass as bass
    import concourse.mybir as mybir
    import concourse.tile as tile
    from concourse import bacc

    F32 = mybir.dt.float32
    F32R = mybir.dt.float32r
    ADD = mybir.AluOpType.add
    MULT = mybir.AluOpType.mult
    EXP = mybir.ActivationFunctionType.Exp

    nc = bacc.Bacc("TRN2", target_bir_lowering=False, num_devices=NCORES)

    xT = nc.dram_tensor("xT", [D, T], F32R, kind="ExternalInput")
    wqk = nc.dram_tensor("wqk", [D, 2 * DIN], F32R, kind="ExternalInput")
    wv = nc.dram_tensor("wv", [D, DIN], F32R, kind="ExternalInput")
    wproj = nc.dram_tensor("wproj", [DIN, D], F32R, kind="ExternalInput")
    bqk = nc.dram_tensor("bqk", [128, 4], F32, kind="ExternalInput")
    bv = nc.dram_tensor("bv", [128, DIN], F32, kind="ExternalInput")
    cos2 = nc.dram_tensor("cos2", [128, T], F32, kind="ExternalInput")
    sin2 = nc.dram_tensor("sin2", [128, T], F32, kind="ExternalInput")
    perm = nc.dram_tensor("perm", [128, 128], F32R, kind="ExternalInput")
    trimask = nc.dram_tensor("trimask", [128, 128], F32, kind="ExternalInput")
    vconst = nc.dram_tensor("vconst", [128, 64], F32R, kind="ExternalInput")
    yT = nc.dram_tensor("yT", [D, T], F32, kind="ExternalOutput")
    if debug:
        dbg_qk = nc.dram_tensor("dbg_qk", [128, 4, T], F32, kind="ExternalOutput")
        dbg_v = nc.dram_tensor("dbg_v", [128, NKT, 2, 192], F32, kind="ExternalOutput")
        if "attn" in stages:
            dbg_y = nc.dram_tensor("dbg_y", [128, 2, T], F32, kind="ExternalOutput")

    with tile.TileContext(nc) as tc:
        with tc.tile_pool(name="const", bufs=1) as constp, \
             tc.tile_pool(name="big", bufs=1) as bigp, \
             tc.tile_pool(name="xin", bufs=2) as xinp, \
             tc.tile_pool(name="ptile", bufs=4) as ptp, \
             tc.tile_pool(name="tmp", bufs=3) as tmpp, \
             tc.tile_pool(name="rsm", bufs=2) as rsmp, \
             tc.tile_pool(name="outs", bufs=3) as outsp, \
             tc.tile_pool(name="psmm", bufs=2, space="PSUM") as psmm, \
             tc.tile_pool(name="pss", bufs=4, space="PSUM") as pss, \
             tc.tile_pool(name="psav", bufs=2, space="PSUM") as psav:

            # ---- persistent SBUF tensors ----
            wqk_sb = constp.tile([128, 8, 2 * DIN], F32R)     # [p, din_o, f]
            wv_sb = constp.tile([128, 8, DIN], F32R)
            wproj_sb = constp.tile([128, 2, D], F32R)         # [p, din_tile, dout]
            bqk_sb = constp.tile([128, 4], F32)
            bv_sb = constp.tile([128, DIN], F32)
            cos_sb = constp.tile([128, T], F32)
            sin_sb = constp.tile([128, T], F32)
            perm_sb = constp.tile([128, 128], F32R)
            tri_sb = constp.tile([128, 128], F32)

            qk_sb = bigp.tile([128, 4, T], F32R)              # fb: q01,q23,k01,k23
            v_sb = bigp.tile([128, NKT, 2, 192], F32R)        # [t_p, kt, hp, cols]
            y_sb = bigp.tile([128, 2, T], F32R)               # y^T (din on partitions)

            nc.sync.dma_start(wqk_sb[:], wqk[:].rearrange("(o p) f -> p o f", p=128))
            nc.sync.dma_start(wv_sb[:], wv[:].rearrange("(o p) f -> p o f", p=128))
            nc.sync.dma_start(wproj_sb[:], wproj[:].rearrange("(o p) f -> p o f", p=128))
            nc.sync.dma_start(bqk_sb[:], bqk[:])
            nc.sync.dma_start(bv_sb[:], bv[:])
            nc.sync.dma_start(cos_sb[:], cos2[:])
            nc.sync.dma_start(sin_sb[:], sin2[:])
            nc.sync.dma_start(perm_sb[:], perm[:])
            nc.sync.dma_start(tri_sb[:], trimask[:])

            # V layout per (kt, hp): [V_even(64) | ones(1) | zeros(63) | V_odd(64)]
            for kt in range(NKT):
                for hp in range(2):
                    nc.sync.dma_start(v_sb[:, kt, hp, 64:128], vconst[:])

            for c in range(NCH):
                cc = bass.ds(c * CHUNK, CHUNK)
                # ---- load x^T chunk ----
                x_sb = xinp.tile([128, 8, CHUNK], F32R, tag="xchunk")
                for o in range(8):
                    nc.sync.dma_start(x_sb[:, o], xT[o * 128:(o + 1) * 128, cc])

                # ---- q^T,k^T for this chunk: [f, t] ----
                for fb in range(4):
                    pq = psmm.tile([128, CHUNK], F32, tag="mm")
                    for o in range(8):
                        nc.tensor.matmul(
                            pq[:], wqk_sb[:, o, fb * 128:(fb + 1) * 128], x_sb[:, o],
                            start=(o == 0), stop=(o == 7))
                    # bias add (per-partition) + round to fp32r
                    nc.vector.tensor_scalar_add(qk_sb[:, fb, cc], pq[:], bqk_sb[:, fb:fb + 1])

                # ---- V for this chunk (natural layout) ----
                for tb in range(4):
                    kt = c * 4 + tb
                    pvfull = psmm.tile([128, CHUNK], F32, tag="mm", name="pvfull")
                    pv = pvfull[:, :DIN]
                    for o in range(8):
                        nc.tensor.matmul(
                            pv[:], x_sb[:, o, tb * 128:(tb + 1) * 128], wv_sb[:, o],
                            start=(o == 0), stop=(o == 7))
                    for l in range(HL):
                        off = 0 if l % 2 == 0 else 128
                        nc.vector.tensor_tensor(
                            v_sb[:, kt, l // 2, off:off + 64],
                            pv[:, l * 64:(l + 1) * 64],
                            bv_sb[:, l * 64:(l + 1) * 64], ADD)

                # ---- RoPE on q^T,k^T chunk (in place) ----
                for fb in range(4 if "rope" in stages else 0):
                    pp = psmm.tile([128, CHUNK], F32, tag="mm")
                    nc.tensor.matmul(pp[:], perm_sb[:], qk_sb[:, fb, cc], start=True, stop=True)
                    swapped = tmpp.tile([128, CHUNK], F32, tag="rope")
                    nc.vector.tensor_tensor(swapped[:], pp[:], sin_sb[:, cc], MULT)
                    nc.vector.tensor_tensor(qk_sb[:, fb, cc], qk_sb[:, fb, cc], cos_sb[:, cc], MULT)
                    nc.vector.tensor_tensor(qk_sb[:, fb, cc], qk_sb[:, fb, cc], swapped[:], ADD)

                # ---- attention for q-chunk c, all local heads ----
                do_s = any(s in stages for s in ("attn", "attn_av", "attn_s"))
                do_av = any(s in stages for s in ("attn", "attn_av"))
                do_norm = "attn" in stages
                nkt_c = 4 * c + 4
                for l in range(HL if do_s else 0):
                    hp, base = l // 2, 64 * (l % 2)
                    q_ap = qk_sb[base:base + 64, hp, cc]
                    if do_av:
                        pav = psav.tile([128, CHUNK], F32, tag="av")
                    p_tiles = []
                    for kt in range(nkt_c):
                        i = kt - 4 * c  # >=0 on diagonal tiles
                        col0 = 128 * i if i >= 0 else 0
                        ps = pss.tile([128, CHUNK], F32, tag="s")
                        nc.tensor.matmul(
                            ps[:], qk_sb[base:base + 64, 2 + hp, kt * 128:(kt + 1) * 128],
                            q_ap, start=True, stop=True)
                        if i >= 0:
                            nc.vector.tensor_tensor(
                                ps[:, col0:col0 + 128], ps[:, col0:col0 + 128], tri_sb[:], ADD)
                        pt = ptp.tile([128, CHUNK], F32R, tag="p")
                        nc.scalar.activation(pt[:, col0:], ps[:, col0:], EXP, bias=0.0, scale=SCALE)
                        p_tiles.append((pt, col0))
                    nout = 65 if l % 2 == 0 else 128
                    for kt in range(nkt_c if do_av else 0):
                        pt, col0 = p_tiles[kt]
                        voff = 0 if l % 2 == 0 else 64
                        nc.tensor.matmul(
                            pav[:nout, col0:], v_sb[:, kt, hp, voff:voff + nout],
                            pt[:, col0:], start=(kt == 0), stop=(kt == nkt_c - 1),
                            skip_group_check=True)
                    if not do_norm:
                        continue
                    # normalize: recip of sums row, gpsimd partition-broadcast
                    srow = 64 if l % 2 == 0 else 0
                    yrows = 0 if l % 2 == 0 else 64
                    r_sb = rsmp.tile([128, CHUNK], F32, tag="r")
                    nc.vector.reciprocal(r_sb[srow:srow + 1, :], pav[srow:srow + 1, :])
                    if "nobc" in stages:
                        nc.vector.tensor_copy(y_sb[base:base + 64, hp, cc], pav[yrows:yrows + 64, :])
                    else:
                        bc_sb = rsmp.tile([128, CHUNK], F32, tag="bcs")
                        src32 = r_sb[srow:srow + 32, :]
                        nc.vector.stream_shuffle(bc_sb[base:base + 32, :], src32, [0] * 32)
                        nc.vector.stream_shuffle(bc_sb[base + 32:base + 64, :], src32, [0] * 32)
                        nc.vector.tensor_tensor(
                            y_sb[base:base + 64, hp, cc], pav[yrows:yrows + 64, :],
                            bc_sb[base:base + 64, :], MULT)

                # ---- partial output projection for chunk c ----
                for db in range(8 if "proj" in stages else 0):
                    pr = psmm.tile([128, CHUNK], F32, tag="mm")
                    for pt2 in range(2):
                        nc.tensor.matmul(
                            pr[:], wproj_sb[:, pt2, db * 128:(db + 1) * 128],
                            y_sb[:, pt2, cc], start=(pt2 == 0), stop=(pt2 == 1))
                    o_sb = outsp.tile([128, CHUNK], F32, tag="o")
                    nc.vector.tensor_copy(o_sb[:], pr[:])
                    nc.sync.dma_start(yT[db * 128:(db + 1) * 128, cc], o_sb[:])

            if debug:
                nc.sync.dma_start(dbg_qk[:], qk_sb[:].bitcast(F32))
                nc.sync.dma_start(dbg_v[:], v_sb[:].bitcast(F32))
                if "attn" in stages:
                    nc.sync.dma_start(dbg_y[:], y_sb[:].bitcast(F32))

    nc.finalize()
    return nc


def _host_inputs(x, Wqkv, bqkv, Wproj):
    """Per-core input maps. Core c: batch c//TP, heads [4*(c%TP), 4*(c%TP)+4)."""
    # RoPE tables in ^T layout, rows = head-local dim d (pattern repeats each 64)
    d = np.arange(64)
    inv_freq = 1.0 / (ROPE_BASE ** (np.arange(0, DH, 2, dtype=np.float64) / DH))  # [32]
    ang = np.arange(T, dtype=np.float64)[None, :] * inv_freq[d // 2][:, None]     # [64, T]
    cos64 = np.cos(ang)
    sin64 = np.sin(ang) * np.where(d % 2 == 0, -1.0, 1.0)[:, None]
    cos2 = np.tile(cos64, (2, 1)).astype(np.float32)
    sin2 = np.tile(sin64, (2, 1)).astype(np.float32)

    perm = np.zeros((128, 128), np.float32)
    perm[np.arange(128) ^ 1, np.arange(128)] = 1.0

    ki, qi = np.meshgrid(np.arange(128), np.arange(128), indexing="ij")
    trimask = np.where(ki <= qi, 0.0, NEG).astype(np.float32)

    vconst_np = np.zeros((128, 64), np.float32)
    vconst_np[:, 0] = 1.0

    Wq, Wk, Wv = Wqkv[:, :D], Wqkv[:, D:2 * D], Wqkv[:, 2 * D:]
    bq, bk, bvv = bqkv[:D], bqkv[D:2 * D], bqkv[2 * D:]

    maps = []
    for core in range(NCORES):
        b, r = core // TP, core % TP
        sl = slice(r * DIN, (r + 1) * DIN)
        wqk_c = np.concatenate([Wq[:, sl], Wk[:, sl]], axis=1)
        bqk_c = np.concatenate([bq[sl], bk[sl]]).astype(np.float32)
        maps.append({
            "xT": _round_fp32r(x[b].T),
            "wqk": _round_fp32r(wqk_c),
            "wv": _round_fp32r(Wv[:, sl]),
            "wproj": _round_fp32r(Wproj[sl, :]),
            "bqk": np.ascontiguousarray(bqk_c.reshape(4, 128).T),
            "bv": np.broadcast_to(bvv[sl].astype(np.float32), (128, DIN)).copy(),
            "cos2": cos2,
            "sin2": sin2,
            "perm": _round_fp32r(perm),
            "trimask": trimask,
            "vconst": vconst_np,
        })
    return maps


def kernel(x, Wqkv, bqkv, Wproj, bproj):
    global _compiled, _last_results
    from concourse.bass_utils import run_bass_kernel_spmd

    if _compiled is None:
        _compiled = _build()
    nc = _compiled

    maps = _host_inputs(
        np.asarray(x, np.float32), np.asarray(Wqkv, np.float32),
        np.asarray(bqkv, np.float32), np.asarray(Wproj, np.float32))
    res = run_bass_kernel_spmd(nc, maps, core_ids=list(range(NCORES)))
    _last_results = res
    out = np.empty((B, T, D), np.float32)
    for b in range(B):
        acc = np.zeros((D, T), np.float64)
        for r in range(TP):
            acc += res.results[b * TP + r]["yT"]
        out[b] = acc.T + np.asarray(bproj, np.float64)[None, :]
    return out
